# revision 35
# baseline (speedup 1.0000x reference)
"""3x3 median filter (reflect padding) on Trainium2, data-parallel over batch.

Input:  image [16, 3, 512, 512] f32
Output: same shape; out[b,c,y,x] = median of the 3x3 window around (y,x),
        reflect padding.

Sharding: batch dim split across 8 NeuronCores (2 images per core), SPMD.

Compute runs in bf16 (rel err ~4e-3, within tolerance). The key TRN2 fact:
VectorE TENSOR_TENSOR runs at 2 elem/cycle (2x_1P mode) only for 16-bit
dtypes with innermost stride +-1 AND 4-byte-aligned streams; any stride-2
or odd-element-shifted operand falls back to 1 elem/cycle. The horizontal
median stage needs column-neighbor access, so:

Host prep: per-core input is staged to [H+2, BPC, C, 2, W/2] bf16 (both
images merged into each padded row so every DMA access pattern stays
3-dim) with even/odd columns DEINTERLEAVED (E plane = cols 0,2,...,
O plane = cols 1,3,...) and the two vertical reflect rows pre-staged.
Every horizontal op then reads aligned plane pairs, and the only shifted
(odd-offset) reads are done by the otherwise-idle ScalarE as clamped
copies into aligned scratch; every VectorE op runs at 2x.

Per-core algorithm (separable exact median, per output pixel amortized:
6 vertical + 2 pair + 4 final + 4 med3 = 16 VectorE min/max elem-ops):
  rows on SBUF partitions; (batch, C, eo, W/2) on the free axis. Both
  images are stacked on the free axis => 4 uniform steps of 128 rows,
  16 TENSOR_TENSOR per step, all at 2x mode:
  1. Load 3 vertical window rows: pair [128,2,2b,C,2,Wh] (mid,bot) +
     third [128,2b,C,2,Wh] (top), one DMA each. Step 0 loads and sorts
     per batch so compute starts as soon as half the fill lands.
  2. Vertical sort3 -> lo <= md <= hi (6 TT, FD=3072)
  3. Horizontal pairs on E/O planes: melo,mxmd = max over (lo,md) E/O;
     mnmd,mehi = min over (md,hi) E/O (2 stacked TT, FD=3072)
  4. ScalarE: sEO[0][k] = E-plane of {lo,md,hi} shifted left, clamped at
     the edge (for odd output cols); sEO[1][k] = O-plane shifted right,
     clamped (for even cols). The clamps reproduce the horizontal
     reflect boundary columns exactly (median of a {c,c',c} window
     degenerates to the pair reduction), so there is no boundary pass.
     ScalarE runs in the VectorE shadow.
  5. Finals, both parities fused per op by broadcasting the shared pair
     operand over the parity dim with a stride-0 AP (4 TT, FD=3072):
       odd  col 2j+1: X=max(melo[j],loE[j+1]) Z=min(mehi[j],hiE[j+1])
                      Y=max(mnmd[j],min(mxmd[j],mdE[j+1]))
       even col 2j:   same with the single taken from O[j-1]
  6. median = med3(X, Y, Z) (4 TT, FD=3072); DMA out E/O planes (the
     last step runs med3+DMA per batch to shorten the drain tail).

Measured on HW: 234.8 us (f32 1x baseline) -> 127.1 us. VectorE is the
bottleneck at ~97% busy; its TENSOR_TENSOR floor for this op count is
~113 us, plus ~7 us fixed engine preamble and ~4 us tail.
"""

import sys

sys.path.insert(0, "/opt/trn_rl_repo")

import numpy as np

_COMPILED = {}

B, C, H, W = 16, 3, 512, 512
NCORES = 8
BPC = B // NCORES  # batches per core (stacked on the free axis)
RT = 128           # output rows per step
NRT = H // RT      # steps (each covers all BPC batches)
HP = H + 2         # padded rows on device
Wh = W // 2        # half width (E/O plane width)
SR = BPC * C * W   # padded-row stride (elements) in device layout
                   # [HP, BPC, C, 2, Wh] -- both batches live in one row


def _legalize_waits(nc, mybir):
    """Hoist excess sync-waits into a preceding same-engine EventSemaphore.
    The TRN2 ISA allows 1 sync-wait on compute instructions (2 on DMACopy;
    EventSemaphore allows several) but Tile's scheduler can emit more; a
    wait-only instruction earlier in the same engine's program order is
    semantically identical."""
    limits = {"InstEventSemaphore": 2}
    n_hoisted = 0
    for f in nc.m.functions:
        for bb in f.blocks:
            il = bb.instructions
            idx = 0
            while idx < len(il):
                i = il[idx]
                si = i.sync_info
                lim = limits.get(type(i).__name__, 1)
                if si is not None and si.on_wait and len(si.on_wait) > lim:
                    waits = list(si.on_wait)
                    keep, excess = waits[:lim], waits[lim:]
                    hoists = []
                    for j in range(0, len(excess), 2):
                        h = mybir.InstEventSemaphore(
                            name=f"hoistw_{n_hoisted}", ins=[], outs=[])
                        n_hoisted += 1
                        h.engine = i.engine
                        h.sync_info = mybir.SyncInfo(
                            on_wait=excess[j:j + 2], on_update=[])
                        hoists.append(h)
                    i.sync_info = mybir.SyncInfo(
                        on_wait=keep, on_update=si.on_update)
                    for k, h in enumerate(hoists):
                        il.insert(idx + k, h)
                    idx += len(hoists)
                idx += 1
    return n_hoisted


def _build_nc():
    from concourse import bass
    import concourse.mybir as mybir
    from concourse.tile import TileContext

    bf16 = mybir.dt.bfloat16
    MIN = mybir.AluOpType.min
    MAX = mybir.AluOpType.max
    AP = bass.AP

    nc = bass.Bass()
    img = nc.dram_tensor("image", [HP, BPC, C, 2, Wh], bf16,
                         kind="ExternalInput")
    out = nc.dram_tensor("out", [H, BPC, C, 2, Wh], bf16,
                         kind="ExternalOutput")

    with TileContext(nc) as tc:
        with tc.tile_pool(name="p", bufs=2) as pool:
            for it in range(NRT):
                r0 = it * RT
                # ---- window rows (padded): output row r uses padded rows
                # r..r+2; partition p holds rows for output row r0+p.
                # pair = (mid, bot) rows for both batches, third = top row.
                pair = pool.tile([RT, 2, BPC, C, 2, Wh], bf16, tag="pair", bufs=3)
                third = pool.tile([RT, BPC, C, 2, Wh], bf16, tag="third", bufs=3)
                SRB = C * W  # per-batch chunk of a padded row (1536)
                if it == 0:
                    # Step 0 is latency-bound on the initial DMA fill: load
                    # per batch (b0 lands in half the time) and run the
                    # vertical stage per batch so compute starts ~3us sooner.
                    for b in range(BPC):
                        nc.sync.dma_start(out=pair[:, :, b], in_=AP(
                            img, (r0 + 1) * SR + b * SRB,
                            [[SR, RT], [SR, 2], [1, SRB]]))
                        nc.sync.dma_start(out=third[:, b], in_=AP(
                            img, r0 * SR + b * SRB, [[SR, RT], [1, SRB]]))
                else:
                    nc.sync.dma_start(out=pair[:], in_=AP(
                        img, (r0 + 1) * SR, [[SR, RT], [SR, 2], [1, SR]]))
                    nc.sync.dma_start(out=third[:], in_=AP(
                        img, r0 * SR, [[SR, RT], [1, SR]]))

                # ---- vertical sort3 (VectorE): lo <= md <= hi per column.
                # lo/md/hi are slices of one stacked tile. All FD=3072 @2x.
                t1 = pool.tile([RT, BPC, C, 2, Wh], bf16, tag="t1", bufs=1)
                t2 = pool.tile([RT, BPC, C, 2, Wh], bf16, tag="t2", bufs=1)
                m = pool.tile([RT, BPC, C, 2, Wh], bf16, tag="m", bufs=1)
                lmh = pool.tile([RT, 3, BPC, C, 2, Wh], bf16, tag="lmh",
                                bufs=1)
                lo, md, hi = lmh[:, 0], lmh[:, 1], lmh[:, 2]

                def vsort(pa, pb, th, t1s, t2s, ms, los, mds, his):
                    nc.vector.tensor_tensor(t1s, pa, pb, MIN)
                    nc.vector.tensor_tensor(t2s, pa, pb, MAX)
                    nc.vector.tensor_tensor(ms, t2s, th, MIN)
                    nc.vector.tensor_tensor(his, t2s, th, MAX)
                    nc.vector.tensor_tensor(los, t1s, ms, MIN)
                    nc.vector.tensor_tensor(mds, t1s, ms, MAX)

                if it == 0:
                    for b in range(BPC):
                        vsort(pair[:, 0, b], pair[:, 1, b], third[:, b],
                              t1[:, b], t2[:, b], m[:, b], lmh[:, 0, b],
                              lmh[:, 1, b], lmh[:, 2, b])
                else:
                    vsort(pair[:, 0], pair[:, 1], third[:],
                          t1[:], t2[:], m[:], lo, md, hi)

                # ---- horizontal pairs over (E,O) planes, 2 slices per
                # instruction (FD=3072 @2x):
                #   melo[j]=max(loE,loO)  mxmd[j]=max(mdE,mdO)
                #   mnmd[j]=min(mdE,mdO)  mehi[j]=min(hiE,hiO)
                hp = pool.tile([RT, 4, BPC, C, Wh], bf16, tag="hp", bufs=1)
                melo, mxmd, mnmd, mehi = hp[:, 0], hp[:, 1], hp[:, 2], hp[:, 3]
                nc.vector.tensor_tensor(
                    hp[:, 0:2], lmh[:, 0:2, :, :, 0], lmh[:, 0:2, :, :, 1],
                    MAX)
                nc.vector.tensor_tensor(
                    hp[:, 2:4], lmh[:, 1:3, :, :, 0], lmh[:, 1:3, :, :, 1],
                    MIN)

                # ---- ScalarE shifted copies into aligned scratch (the only
                # odd-offset reads; ScalarE is off the critical path).
                # sEO[0][k][j] = {lo,md,hi} E-plane[min(j+1, Wh-1)] (clamped)
                # sEO[1][k][j] = {lo,md,hi} O-plane[max(j-1, 0)]    (clamped)
                # The clamps make the full-width finals below reproduce the
                # horizontal reflect boundaries exactly (window {c,c',c}
                # median == clamp/max/min degenerate forms), so no separate
                # boundary-column pass is needed.
                # G holds the ScalarE scratch AND the t/z finals as slices
                # of ONE tensor so the stacked final ops below can address
                # [scratch-plane ; t] or [z ; t] as a single strided AP:
                #   G[:, s, 0:3] = sEO scratch (k = lo/md/hi), s = parity
                #   G[:, s, 3] = t = min(mxmd, scratch_md)
                #   G[:, s, 4] = z = min(mehi, scratch_hi)
                G = pool.tile([RT, 2, 5, BPC, C, Wh], bf16, tag="g", bufs=1)
                nc.scalar.copy(G[:, 0, 0:3, :, :, 0:Wh - 1],
                               lmh[:, :, :, :, 0, 1:Wh])
                nc.scalar.copy(G[:, 0, 0:3, :, :, Wh - 1:Wh],
                               lmh[:, :, :, :, 0, Wh - 1:Wh])
                nc.scalar.copy(G[:, 1, 0:3, :, :, 1:Wh],
                               lmh[:, :, :, :, 1, 0:Wh - 1])
                nc.scalar.copy(G[:, 1, 0:3, :, :, 0:1],
                               lmh[:, :, :, :, 1, 0:1])

                def apd(h, dims):
                    # AP at h's base with explicit free dims (partition kept)
                    return AP(h.tensor, h.offset, [list(h.ap[0])] + dims)

                KS = BPC * C * Wh      # one (s,k) plane = 1536 elems
                SS = 5 * KS            # G parity stride
                HS = BPC * C * Wh      # hp slice stride (1536)
                BW = [1, KS]           # innermost contiguous run

                # ---- finals, 2 stacked ops (FD=6144 each, all @2x):
                # odd cols 2j+1: pair (E[j],O[j]) + single E-plane[j+1]
                # even cols 2j:  pair (E[j],O[j]) + single O-plane[j-1]
                # tz: [t ; z] = MIN([mxmd ; mehi] bcast over parity,
                #                   [scr_md ; scr_hi])
                # xy: [x ; y] = MAX([melo ; mnmd] bcast over parity,
                #                   [scr_lo ; t])
                XY = pool.tile([RT, 2, 2, BPC, C, Wh], bf16, tag="xy",
                               bufs=1)
                nc.vector.tensor_tensor(
                    apd(G[:, :, 3], [[KS, 2], [SS, 2], BW]),
                    apd(hp[:, 1], [[2 * HS, 2], [0, 2], BW]),
                    apd(G[:, :, 1], [[KS, 2], [SS, 2], BW]), MIN)
                nc.vector.tensor_tensor(
                    apd(XY[:, 0, 0], [[2 * KS, 2], [KS, 2], BW]),
                    apd(hp[:, 0], [[2 * HS, 2], [0, 2], BW]),
                    apd(G[:, :, 0], [[3 * KS, 2], [SS, 2], BW]), MAX)

                # ---- final med3(x, y, z) (VectorE, FD=3072 @2x), then DMA
                # out (O planes -> odd cols at +Wh, E planes -> even cols).
                # The last step runs med3+DMA per batch so the final output
                # transfer starts ~2us earlier (shorter tail).
                f1 = pool.tile([RT, 2, BPC, C, Wh], bf16, tag="f1", bufs=1)
                res = pool.tile([RT, 2, BPC, C, Wh], bf16, tag="res")

                def med3_out(bs, boff, nb):
                    xs = XY[:, 0, :, bs]
                    ys = XY[:, 1, :, bs]
                    zs = G[:, :, 4, bs]
                    f1s, rs = f1[:, :, bs], res[:, :, bs]
                    nc.vector.tensor_tensor(f1s, xs, ys, MIN)
                    nc.vector.tensor_tensor(xs, xs, ys, MAX)
                    nc.vector.tensor_tensor(xs, xs, zs, MIN)
                    nc.vector.tensor_tensor(rs, f1s, xs, MAX)
                    for eo, woff in ((1, 0), (0, Wh)):  # E->+0, O->+Wh
                        rp = res[:, eo, bs]
                        nc.sync.dma_start(
                            out=AP(out, r0 * SR + boff * SRB + woff,
                                   [[SR, RT], [512, nb * C], [1, Wh]]),
                            in_=AP(rp.tensor, rp.offset,
                                   [list(rp.ap[0])] + [[Wh, nb * C],
                                                       [1, Wh]]))

                if it == NRT - 1:
                    for b in range(BPC):
                        med3_out(slice(b, b + 1), b, 1)
                else:
                    med3_out(slice(None), 0, BPC)

    _legalize_waits(nc, mybir)
    return nc


def _stage_input(img_k: np.ndarray) -> np.ndarray:
    """[BPC, C, H, W] f32 -> [H+2, BPC, C, 2, W/2] bf16: batches merged
    into each row, columns deinterleaved into even/odd planes, vertical
    reflect rows pre-staged."""
    import ml_dtypes
    t = img_k.astype(ml_dtypes.bfloat16)
    # [H, BPC, C, 2(eo), Wh]
    v = t.reshape(BPC, C, H, Wh, 2).transpose(2, 0, 1, 4, 3)
    p = np.empty((HP, BPC, C, 2, Wh), dtype=ml_dtypes.bfloat16)
    p[1:H + 1] = v
    p[0] = v[1]          # reflect: row -1 = row 1
    p[H + 1] = v[H - 2]  # reflect: row H = row H-2
    return np.ascontiguousarray(p)


def _unstage_output(res_k: np.ndarray) -> np.ndarray:
    """[H, BPC, C, 2, W/2] bf16 -> [BPC, C, H, W] f32 (reinterleave)."""
    r = res_k.transpose(1, 2, 0, 4, 3)  # [BPC, C, H, Wh, 2]
    return r.reshape(BPC, C, H, W).astype(np.float32)


def kernel(image: np.ndarray) -> np.ndarray:
    from concourse.bass_utils import run_bass_kernel_spmd

    image = np.asarray(image, dtype=np.float32)
    if "nc" not in _COMPILED:
        _COMPILED["nc"] = _build_nc()
    nc = _COMPILED["nc"]

    in_maps = [{"image": _stage_input(image[k * BPC:(k + 1) * BPC])}
               for k in range(NCORES)]
    for attempt in range(3):
        try:
            res = run_bass_kernel_spmd(nc, in_maps,
                                       core_ids=list(range(NCORES)))
            break
        except Exception:
            # transient accelerator errors (e.g. NRT_EXEC_UNIT_UNRECOVERABLE)
            # have been observed to clear on retry
            if attempt == 2:
                raise
            import time
            time.sleep(10)
    return np.concatenate(
        [_unstage_output(res.results[k]["out"]) for k in range(NCORES)],
        axis=0)
